# revision 14
# baseline (speedup 1.0000x reference)
"""BottleneckLSTMCell fused kernel for 8 Trainium2 NeuronCores.

Sharding: data-parallel over batch (B=8 -> 1 image per core). Each core runs
the full cell for its image:

  phase A: xw = dw3x3(x) (+bias folded into the Wy bias); i = Wy @ [h; xw] + b
  phase B: b = dw3x3(i); four 1x1 gate matmuls; LSTM pointwise -> (ch, cc)

All matmul traffic is float16 (fp32 PSUM accumulate): same 1 col/cycle PE
stream rate as float32r but FWL-accelerated weight loads, half the DMA bytes
and half the SBUF footprint. The depthwise convs run as 9 per-tap diagonal
matmuls on the tensor engine, except dw-i chunks listed in DWI_DVE which run
on the vector engine as per-channel-scalar multiply (tensor_scalar, 4x mode)
+ accumulate (tensor_tensor, 2x mode) chains, reading from a pair of
1-px-shifted fp16 i images (P0/P1) so every tap is 4B-aligned. Gate psums are
evacuated in [128,2,512] batches (paired sigmoid in one ACT op); the LSTM
pointwise runs on DVE in fp16. Outputs return as fp16 and are upcast on host.
"""

import sys

if '/opt/trn_rl_repo' not in sys.path:
    sys.path.insert(0, '/opt/trn_rl_repo')

import numpy as np

import concourse.bass as bass  # noqa: F401
from concourse import bacc
import concourse.mybir as mybir
from concourse.tile import TileContext
from concourse.bass_utils import run_bass_kernel_spmd

F32 = mybir.dt.float32
F16 = mybir.dt.float16
AF = mybir.ActivationFunctionType
ALU = mybir.AluOpType

B, CIN, CH, HW = 8, 320, 512, 64
PIX = HW * HW          # 4096
NCORES = 8
NCHUNK = 8             # spatial slabs of 8 rows (512 px)
XCH = [128, 128, 64]   # x channel chunk sizes (320)
DWI_DVE = (2, 3)       # dw-i channel chunks computed on the vector engine
DWI_PE = tuple(m for m in range(4) if m not in DWI_DVE)
DWX_DVE = True         # dw-x chunk 2 (64 ch) on the vector engine
TV_ON_GP = True        # f*c pointwise product on GpSimd

taps = [(t // 3 - 1, t % 3 - 1) for t in range(9)]


def build_nc():
    nc = bacc.Bacc(None, target_bir_lowering=False, num_devices=NCORES)

    xd = nc.dram_tensor("x", (CIN, 66, 66), F16, kind="ExternalInput")
    hd = nc.dram_tensor("h", (CH, PIX), F16, kind="ExternalInput")
    cd = nc.dram_tensor("c", (CH, PIX), F16, kind="ExternalInput")
    wyd = nc.dram_tensor("wy", (128, 7, 512), F16, kind="ExternalInput")
    wybd = nc.dram_tensor("wyb", (128, 4), F32, kind="ExternalInput")
    wgd = nc.dram_tensor("wg", (128, 16, 512), F16, kind="ExternalInput")
    dwxd = nc.dram_tensor("dwx", (128, 3, 1152), F16, kind="ExternalInput")
    dwid = nc.dram_tensor("dwi", (128, len(DWI_PE), 1152), F16,
                          kind="ExternalInput")
    wivd = nc.dram_tensor("wiv", (128, 9 * len(DWI_DVE)), F32,
                          kind="ExternalInput")
    wxvd = nc.dram_tensor("wxv", (128, 9), F32, kind="ExternalInput")
    xv0d = nc.dram_tensor("xv0", (64, 66, 68), F16, kind="ExternalInput")
    xv1d = nc.dram_tensor("xv1", (64, 66, 68), F16, kind="ExternalInput")
    zd = nc.dram_tensor("zz", (128, 128), F16, kind="ExternalInput")
    ccd = nc.dram_tensor("occ", (CH, PIX), F16, kind="ExternalOutput")
    chd = nc.dram_tensor("och", (CH, PIX), F16, kind="ExternalOutput")

    x_ap, h_ap, c_ap = xd.ap(), hd.ap(), cd.ap()
    cc_ap, ch_ap = ccd.ap(), chd.ap()

    with TileContext(nc) as tc:
        with tc.tile_pool(name="persist", bufs=1) as pp, \
             tc.tile_pool(name="wB", bufs=1) as wB:
            # small zeros tile used to paint halo borders
            zt = pp.tile([128, 68], F16, tag="zt", name="zt")
            nc.sync.dma_start(out=zt[:, 0:68], in_=zd.ap()[:, 0:68])

            # i image, fp16. P0: pixel (r,c) at [r+1, c+2] (so the dx=0 tap
            # slab reads start 4B-aligned). P1 (DVE chunks only): pixel at
            # [r+1, c+1] = P0 shifted left one px, making dx=+-1 taps
            # 4B-aligned too.
            i_p0 = [pp.tile([128, 66, 68], F16, tag=f"ip0{m}", name=f"ip0{m}")
                    for m in range(4)]
            i_p1 = {m: pp.tile([128, 66, 68], F16, tag=f"ip1{m}",
                               name=f"ip1{m}") for m in DWI_DVE}
            for m in range(4):
                nc.scalar.copy(i_p0[m][:, 0, :], zt[:, :])
                nc.scalar.copy(i_p0[m][:, 65, :], zt[:, :])
                nc.scalar.copy(i_p0[m][:, :, 1], zt[:, :66])
                nc.scalar.copy(i_p0[m][:, :, 66], zt[:, :66])
            for m in DWI_DVE:
                nc.scalar.copy(i_p1[m][:, 0, :], zt[:, :])
                nc.scalar.copy(i_p1[m][:, 65, :], zt[:, :])
                nc.scalar.copy(i_p1[m][:, :, 0], zt[:, :66])
                nc.scalar.copy(i_p1[m][:, :, 65], zt[:, :66])

            # ---------------- phase A ----------------
            with (
                tc.tile_pool(name="wA", bufs=1) as wA,
                tc.tile_pool(name="sA", bufs=2) as sA,
                tc.tile_pool(name="psxw", bufs=3, space="PSUM") as psxw,
                tc.tile_pool(name="psi", bufs=4, space="PSUM") as psi,
            ):
                def emit_slab_inputs(n):
                    r0 = 8 * n
                    xpads = []
                    npe = 2 if DWX_DVE else 3
                    for ci in range(npe):
                        pc = XCH[ci]
                        xp = sA.tile([128, 10, 66], F16, tag=f"xpad{ci}",
                                     name=f"xpad{ci}")
                        nc.sync.dma_start(
                            out=xp[:pc, :, :],
                            in_=x_ap[128 * ci:128 * ci + pc, r0:r0 + 10, :],
                        )
                        xpads.append(xp)
                    if DWX_DVE:
                        xv = []
                        for nm, dram in (("xv0", xv0d), ("xv1", xv1d)):
                            t = sA.tile([64, 10, 68], F16, tag=nm, name=nm)
                            nc.sync.dma_start(
                                out=t[:, :, :], in_=dram.ap()[:, r0:r0 + 10, :])
                            xv.append(t)
                        xpads.append(xv)
                    h_sb = []
                    for k in range(4):
                        t = sA.tile([128, 512], F16, tag=f"h{k}", name=f"h{k}")
                        nc.sync.dma_start(
                            out=t[:],
                            in_=h_ap[128 * k:128 * (k + 1), 512 * n:512 * (n + 1)],
                        )
                        h_sb.append(t)
                    return h_sb, xpads

                # startup-critical first: dw-x chunk-0 weights + slab 0
                # inputs so the first matmuls start as early as possible
                dwx_t = wA.tile([128, 3, 1152], F16, tag="dwx", name="dwx")
                nc.sync.dma_start(out=dwx_t[:, 0, :], in_=dwxd.ap()[:, 0, :])
                early = {0: emit_slab_inputs(0)}
                for _ci in (1, 2):
                    nc.sync.dma_start(out=dwx_t[:, _ci, :], in_=dwxd.ap()[:, _ci, :])
                wy_t = wA.tile([128, 7, 512], F16, tag="wy", name="wy")
                for _k in range(7):
                    nc.sync.dma_start(out=wy_t[:, _k, :], in_=wyd.ap()[:, _k, :])
                wyb_t = wA.tile([128, 4], F32, tag="wyb", name="wyb")
                nc.sync.dma_start(out=wyb_t[:], in_=wybd.ap())
                wxv_t = wA.tile([128, 9], F32, tag="wxv", name="wxv")
                nc.sync.dma_start(out=wxv_t[:], in_=wxvd.ap())
                early[1] = emit_slab_inputs(1)
                # prefetch phase-B weights while phase A computes
                wg_t = wB.tile([128, 16, 512], F16, tag="wg", name="wg")
                for _k in range(16):
                    nc.sync.dma_start(out=wg_t[:, _k, :], in_=wgd.ap()[:, _k, :])
                dwi_t = wB.tile([128, len(DWI_PE), 1152], F16, tag="dwi",
                                name="dwi")
                for _ci in range(len(DWI_PE)):
                    nc.sync.dma_start(out=dwi_t[:, _ci, :], in_=dwid.ap()[:, _ci, :])
                wiv_t = wB.tile([128, 9 * len(DWI_DVE)], F32, tag="wiv",
                                name="wiv")
                nc.sync.dma_start(out=wiv_t[:], in_=wivd.ap())

                for n in range(NCHUNK):
                    r0 = 8 * n
                    if n in early:
                        h_sb, xpads = early[n]
                    else:
                        h_sb, xpads = emit_slab_inputs(n)

                    # depthwise 3x3 on x: 9 diag matmuls per chunk -> PSUM
                    # (chunk 2 on the vector engine when DWX_DVE)
                    xw_sb = []
                    npe = 2 if DWX_DVE else 3
                    for ci in range(npe):
                        pc = XCH[ci]
                        ps = psxw.tile([128, 8, 64], F32, tag="psxw", name="psxw")
                        for t, (dy, dx) in enumerate(taps):
                            nc.tensor.matmul(
                                ps[:pc, :, :],
                                dwx_t[:pc, ci, 128 * t:128 * t + pc],
                                xpads[ci][:pc, 1 + dy:9 + dy, 1 + dx:65 + dx],
                                start=(t == 0),
                                stop=(t == 8),
                            )
                        xw = sA.tile([128, 512], F16, tag=f"xw{ci}",
                                     name=f"xw{ci}", bufs=1)
                        nc.scalar.copy(xw[:pc, :], ps[:pc, :, :])
                        xw_sb.append(xw)
                    if DWX_DVE:
                        xv0_t, xv1_t = xpads[2]
                        xaccs = [
                            sA.tile([64, 8, 64], F16, tag=f"xacc{p}",
                                    name=f"xacc{p}")
                            for p in range(2)
                        ]
                        xtmp = sA.tile([64, 8, 64], F16, tag="xtmp",
                                       name="xtmp")
                        for t, (dy, dx) in enumerate(taps):
                            if dx == 0:
                                src = xv0_t[:, 1 + dy:9 + dy, 2:66]
                            elif dx == -1:
                                src = xv1_t[:, 1 + dy:9 + dy, 0:64]
                            else:
                                src = xv1_t[:, 1 + dy:9 + dy, 2:66]
                            w_ap = wxv_t[:64, t:t + 1]
                            if t == 0:
                                nc.vector.tensor_scalar(
                                    out=xaccs[0][:, :, :], in0=src,
                                    scalar1=w_ap, scalar2=None, op0=ALU.mult)
                            else:
                                nc.vector.tensor_scalar(
                                    out=xtmp[:, :, :], in0=src,
                                    scalar1=w_ap, scalar2=None, op0=ALU.mult)
                                nc.vector.tensor_tensor(
                                    out=xaccs[t % 2][:, :, :],
                                    in0=xaccs[(t + 1) % 2][:, :, :],
                                    in1=xtmp[:, :, :], op=ALU.add)
                        xw_sb.append(xaccs[0])  # tap 8 lands in xaccs[0]

                    # i = Wy @ [h; xw] + bias -> i_p0 (and i_p1) interior
                    for m in range(4):
                        ps = psi.tile([128, 512], F32, tag="psi", name="psi")
                        for k in range(4):  # h chunks first (ready earlier)
                            nc.tensor.matmul(
                                ps[:, :],
                                wy_t[:, k, 128 * m:128 * (m + 1)],
                                h_sb[k][:, :],
                                start=(k == 0),
                                stop=False,
                            )
                        for j in range(3):
                            pc = XCH[j]
                            rhs = (xw_sb[j][:, :, :] if (DWX_DVE and j == 2)
                                   else xw_sb[j][:pc, :])
                            nc.tensor.matmul(
                                ps[:, :],
                                wy_t[:pc, 4 + j, 128 * m:128 * (m + 1)],
                                rhs,
                                start=False,
                                stop=(j == 2),
                            )
                        nc.scalar.activation(
                            i_p0[m][:, 1 + r0:9 + r0, 2:66],
                            ps[:, :],
                            AF.Identity,
                            bias=wyb_t[:, m:m + 1],
                            scale=1.0,
                        )
                        if m in DWI_DVE:
                            nc.scalar.activation(
                                i_p1[m][:, 1 + r0:9 + r0, 1:65],
                                ps[:, :],
                                AF.Identity,
                                bias=wyb_t[:, m:m + 1],
                                scale=1.0,
                            )

            # ---------------- phase B ----------------
            with (
                tc.tile_pool(name="sB", bufs=2) as sB,
                tc.tile_pool(name="psb", bufs=2, space="PSUM") as psb,
                tc.tile_pool(name="psg", bufs=3, space="PSUM") as psg,
            ):
                for n in range(NCHUNK):
                    r0 = 8 * n
                    b_sb = [None] * 4
                    # dw-i on tensor engine for chunks in DWI_PE
                    for idx, m in enumerate(DWI_PE):
                        ps = psb.tile([128, 8, 64], F32, tag="psb", name="psb")
                        for t, (dy, dx) in enumerate(taps):
                            nc.tensor.matmul(
                                ps[:, :, :],
                                dwi_t[:, idx, 128 * t:128 * (t + 1)],
                                i_p0[m][:, 1 + r0 + dy:9 + r0 + dy,
                                        2 + dx:66 + dx],
                                start=(t == 0),
                                stop=(t == 8),
                            )
                        bt = sB.tile([128, 8, 64], F16, tag=f"b{m}",
                                     name=f"b{m}")
                        nc.scalar.copy(bt[:, :, :], ps[:, :, :])
                        b_sb[m] = bt
                    # dw-i on vector engine for chunks in DWI_DVE:
                    # per tap: t = in * w[ch]  (tensor_scalar, 4x fp16)
                    #          acc = acc + t   (tensor_tensor, 2x fp16)
                    for di, m in enumerate(DWI_DVE):
                        accs = [
                            sB.tile([128, 8, 64], F16, tag=f"bacc{m}{p}",
                                    name=f"bacc{m}{p}")
                            for p in range(2)
                        ]
                        tmp = sB.tile([128, 8, 64], F16, tag=f"btmp{m}",
                                      name=f"btmp{m}")
                        for t, (dy, dx) in enumerate(taps):
                            if dx == 0:
                                src = i_p0[m][:, 1 + r0 + dy:9 + r0 + dy, 2:66]
                            elif dx == -1:
                                src = i_p1[m][:, 1 + r0 + dy:9 + r0 + dy, 0:64]
                            else:
                                src = i_p1[m][:, 1 + r0 + dy:9 + r0 + dy, 2:66]
                            w_ap = wiv_t[:, 9 * di + t:9 * di + t + 1]
                            if t == 0:
                                nc.vector.tensor_scalar(
                                    out=accs[0][:, :, :], in0=src,
                                    scalar1=w_ap, scalar2=None, op0=ALU.mult)
                            else:
                                nc.vector.tensor_scalar(
                                    out=tmp[:, :, :], in0=src,
                                    scalar1=w_ap, scalar2=None, op0=ALU.mult)
                                nc.vector.tensor_tensor(
                                    out=accs[t % 2][:, :, :],
                                    in0=accs[(t + 1) % 2][:, :, :],
                                    in1=tmp[:, :, :], op=ALU.add)
                        b_sb[m] = accs[0]  # tap 8 lands in accs[0]

                    for m in range(4):
                        c_t = sB.tile([128, 512], F16, tag="c", name="c")
                        nc.sync.dma_start(
                            out=c_t[:],
                            in_=c_ap[128 * m:128 * (m + 1), 512 * n:512 * (n + 1)],
                        )
                        # gate order in wg packing: pair0 = (i, f) both
                        # sigmoid; pair1 = (o, c) sigmoid + relu
                        pstiles = []
                        for gp in range(2):
                            ps = psg.tile([128, 2, 512], F32, tag="psg",
                                          name="psg")
                            for gg in range(2):
                                g = 2 * gp + gg
                                for k in range(4):
                                    nc.tensor.matmul(
                                        ps[:, gg, :],
                                        wg_t[:, 4 * g + k, 128 * m:128 * (m + 1)],
                                        b_sb[k][:, :, :],
                                        start=(k == 0),
                                        stop=(k == 3),
                                    )
                            pstiles.append(ps)
                        sif = sB.tile([128, 2, 512], F16, tag="sif", name="sif")
                        nc.scalar.activation(sif[:, :, :], pstiles[0][:, :, :],
                                             AF.Sigmoid)
                        so = sB.tile([128, 512], F16, tag="so", name="so")
                        nc.scalar.activation(so[:, :], pstiles[1][:, 0, :],
                                             AF.Sigmoid)
                        gcr = sB.tile([128, 512], F16, tag="gcr", name="gcr")
                        nc.scalar.activation(gcr[:, :], pstiles[1][:, 1, :],
                                             AF.Relu)

                        # pointwise, fp16 on DVE:
                        # cc = f*c + i*min(gc,6); ch = o*clip(cc,0,6)
                        t6 = sB.tile([128, 512], F16, tag="t6", name="t6")
                        nc.vector.tensor_scalar(
                            out=t6[:, :], in0=gcr[:, :],
                            scalar1=6.0, scalar2=None, op0=ALU.min)
                        u = sB.tile([128, 512], F16, tag="u", name="u")
                        nc.vector.tensor_mul(u[:, :], t6[:, :], sif[:, 0, :])
                        tv = sB.tile([128, 512], F16, tag="tv", name="tv")
                        if TV_ON_GP:
                            nc.gpsimd.tensor_tensor(
                                out=tv[:, :], in0=sif[:, 1, :],
                                in1=c_t[:, :], op=ALU.mult)
                        else:
                            nc.vector.tensor_mul(tv[:, :], sif[:, 1, :],
                                                 c_t[:, :])
                        cc_t = sB.tile([128, 512], F16, tag="cc", name="cc")
                        nc.vector.tensor_add(cc_t[:, :], u[:, :], tv[:, :])
                        nc.sync.dma_start(
                            out=cc_ap[128 * m:128 * (m + 1), 512 * n:512 * (n + 1)],
                            in_=cc_t[:],
                        )
                        rcc = sB.tile([128, 512], F16, tag="rcc", name="rcc")
                        nc.vector.tensor_scalar(
                            out=rcc[:, :], in0=cc_t[:, :],
                            scalar1=0.0, scalar2=6.0, op0=ALU.max, op1=ALU.min)
                        ch_t = sB.tile([128, 512], F16, tag="ch", name="ch")
                        nc.vector.tensor_mul(ch_t[:, :], rcc[:, :], so[:, :])
                        nc.sync.dma_start(
                            out=ch_ap[128 * m:128 * (m + 1), 512 * n:512 * (n + 1)],
                            in_=ch_t[:],
                        )

    nc.compile()
    return nc


def pack_weights(W_dw, W_dwb, Wy, Wy_b, Wi, Wbi, Wbf, Wbc, Wbo):
    WyT = Wy[:, :, 0, 0].T.astype(np.float32)  # (832, 512) lhsT
    wy = np.zeros((128, 7, 512), np.float32)
    for k in range(4):  # h chunks first
        wy[:, k, :] = WyT[320 + 128 * k:320 + 128 * (k + 1), :]
    for k in range(2):
        wy[:, 4 + k, :] = WyT[128 * k:128 * (k + 1), :]
    wy[:64, 6, :] = WyT[256:320, :]

    wyb = (Wy_b + Wy[:, :320, 0, 0] @ W_dwb).astype(np.float32)
    wyb = wyb.reshape(4, 128).T.copy()

    def diag_pack(Wtaps, nch, chunks):
        out = np.zeros((128, len(chunks), 1152), np.float32)
        w = Wtaps[:, 0].reshape(nch, 9)  # (nch, 9) tap-major (dy,dx)
        for oi, ci in enumerate(chunks):
            pc = min(128, nch - 128 * ci)
            for t in range(9):
                idx = np.arange(pc)
                out[idx, oi, 128 * t + idx] = w[128 * ci + idx, t]
        return out

    dwx = diag_pack(W_dw, CIN, (0, 1, 2))
    dwi = diag_pack(Wi, CH, DWI_PE)

    wiv = np.zeros((128, 9 * len(DWI_DVE)), np.float32)
    wi9 = Wi[:, 0].reshape(CH, 9)
    for di, m in enumerate(DWI_DVE):
        wiv[:, 9 * di:9 * (di + 1)] = wi9[128 * m:128 * (m + 1), :]

    wxv = np.zeros((128, 9), np.float32)
    wxv[:64, :] = W_dw[256:320, 0].reshape(64, 9)

    # gate order (i, f, o, c): pair0 sigmoid-sigmoid, pair1 sigmoid-relu
    wg = np.zeros((128, 16, 512), np.float32)
    for g, W in enumerate([Wbi, Wbf, Wbo, Wbc]):
        lhsT = W[:, :, 0, 0].T.astype(np.float32)  # (512 in, 512 out)
        for k in range(4):
            wg[:, 4 * g + k, :] = lhsT[128 * k:128 * (k + 1), :]

    f16 = lambda a: np.ascontiguousarray(a, dtype=np.float16)
    return {
        "wy": f16(wy), "wyb": np.ascontiguousarray(wyb), "wg": f16(wg),
        "dwx": f16(dwx), "dwi": f16(dwi), "wiv": np.ascontiguousarray(wiv),
        "wxv": np.ascontiguousarray(wxv),
    }


_CACHE = {}


def _get_nc():
    if "nc" not in _CACHE:
        _CACHE["nc"] = build_nc()
    return _CACHE["nc"]


def run(inputs, trace=False, tmpdir=None):
    """inputs: dict as from setup_inputs(). Returns ((ch, cc), results_obj)."""
    inp = {k: np.asarray(v, np.float32) for k, v in inputs.items()}
    packed = pack_weights(
        inp["W_dw"], inp["W_dwb"], inp["Wy"], inp["Wy_b"], inp["Wi"],
        inp["Wbi"], inp["Wbf"], inp["Wbc"], inp["Wbo"],
    )
    xpad_host = np.zeros((B, CIN, 66, 66), np.float16)
    xpad_host[:, :, 1:65, 1:65] = inp["x"]
    # dw-x DVE chunk: x channels 256:320 in the aligned P0/P1 layouts
    xv0_host = np.zeros((B, 64, 66, 68), np.float16)
    xv0_host[:, :, 1:65, 2:66] = inp["x"][:, 256:320]
    xv1_host = np.zeros((B, 64, 66, 68), np.float16)
    xv1_host[:, :, 1:65, 1:65] = inp["x"][:, 256:320]
    h_host = inp["h"].reshape(B, CH, PIX).astype(np.float16)
    c_host = inp["c"].reshape(B, CH, PIX).astype(np.float16)
    in_maps = []
    for b in range(B):
        in_maps.append({
            "x": xpad_host[b],
            "xv0": xv0_host[b],
            "xv1": xv1_host[b],
            "h": np.ascontiguousarray(h_host[b]),
            "c": np.ascontiguousarray(c_host[b]),
            "zz": np.zeros((128, 128), np.float16),
            **packed,
        })
    nc = _get_nc()
    kwargs = {}
    if trace:
        _enable_trace_hooks()
        kwargs = dict(trace=True, trace_cores=[0])
        if tmpdir:
            kwargs["tmpdir"] = tmpdir
    res = run_bass_kernel_spmd(nc, in_maps, core_ids=list(range(NCORES)), **kwargs)
    ch = np.stack([res.results[b]["och"].reshape(CH, HW, HW) for b in range(B)])
    cc = np.stack([res.results[b]["occ"].reshape(CH, HW, HW) for b in range(B)])
    return (ch.astype(np.float32), cc.astype(np.float32)), res


def kernel(**inputs):
    (ch, cc), _ = run(inputs, trace=False)
    return ch, cc


# ---------- optional NTFF tracing support (test harness only) ----------

def _enable_trace_hooks():
    import types, ctypes, contextlib
    if "antenv.axon_hooks" in sys.modules:
        return
    import concourse.bass_utils as bass_utils

    def _ntff_profile_via_ctypes(so_path):
        lib = ctypes.CDLL(so_path)
        if not hasattr(lib, "axon_start_nrt_profile"):
            return None
        lib.axon_start_nrt_profile.argtypes = [
            ctypes.POINTER(ctypes.c_int64), ctypes.c_size_t]
        lib.axon_start_nrt_profile.restype = ctypes.c_int64
        lib.axon_stop_nrt_profile.argtypes = [ctypes.c_char_p]
        lib.axon_stop_nrt_profile.restype = ctypes.c_int64

        @contextlib.contextmanager
        def _hook(output_dir, device_ids):
            import jax
            jax.devices()
            if device_ids:
                ids = (ctypes.c_int64 * len(device_ids))(*device_ids)
                rc = lib.axon_start_nrt_profile(ids, len(device_ids))
            else:
                rc = lib.axon_start_nrt_profile(None, 0)
            if rc != 0:
                raise RuntimeError(f"axon_start_nrt_profile rc={rc}")
            try:
                yield
            finally:
                lib.axon_stop_nrt_profile(str(output_dir).encode())
        return _hook

    hook = _ntff_profile_via_ctypes("/opt/axon/libaxon_pjrt.so")
    mod = types.ModuleType("antenv.axon_hooks")
    mod.get_axon_ntff_profile_hook = lambda: hook
    mod.set_axon_ntff_profile_hook = lambda h: None
    sys.modules["antenv.axon_hooks"] = mod
    bass_utils.upload_artifacts = lambda tmpdir: "local://" + str(tmpdir)


# revision 20
# speedup vs baseline: 1.0312x; 1.0312x over previous
"""BottleneckLSTMCell fused kernel for 8 Trainium2 NeuronCores.

Sharding: data-parallel over batch (B=8 -> 1 image per core). Each core runs
the full cell for its image:

  phase A: xw = dw3x3(x) (+bias folded into the Wy bias); i = Wy @ [h; xw] + b
  phase B: b = dw3x3(i); four 1x1 gate matmuls; LSTM pointwise -> (ch, cc)

All matmul traffic is float16 (fp32 PSUM accumulate): same 1 col/cycle PE
stream rate as float32r but FWL-accelerated weight loads, half the DMA bytes
and half the SBUF footprint. The depthwise convs run as 9 per-tap diagonal
matmuls on the tensor engine, except dw-i chunks listed in DWI_DVE which run
on the vector engine as per-channel-scalar multiply (tensor_scalar, 4x mode)
+ accumulate (tensor_tensor, 2x mode) chains, reading from a pair of
1-px-shifted fp16 i images (P0/P1) so every tap is 4B-aligned. Gate psums are
evacuated in [128,2,512] batches (paired sigmoid in one ACT op); the LSTM
pointwise runs on DVE in fp16. Outputs return as fp16 and are upcast on host.
"""

import sys

if '/opt/trn_rl_repo' not in sys.path:
    sys.path.insert(0, '/opt/trn_rl_repo')

import numpy as np

import concourse.bass as bass  # noqa: F401
from concourse import bacc
import concourse.mybir as mybir
from concourse.tile import TileContext
from concourse.bass_utils import run_bass_kernel_spmd

F32 = mybir.dt.float32
F16 = mybir.dt.float16
AF = mybir.ActivationFunctionType
ALU = mybir.AluOpType

B, CIN, CH, HW = 8, 320, 512, 64
PIX = HW * HW          # 4096
NCORES = 8
NCHUNK = 8             # spatial slabs of 8 rows (512 px)
XCH = [128, 128, 64]   # x channel chunk sizes (320)
DWI_DVE = (2, 3)       # dw-i channel chunks computed on the vector engine
DWI_PE = tuple(m for m in range(4) if m not in DWI_DVE)
DWX_DVE = False        # dw-x chunk 2 (64 ch) on the vector engine
TV_ON_GP = False       # f*c pointwise product on GpSimd

taps = [(t // 3 - 1, t % 3 - 1) for t in range(9)]


def build_nc():
    nc = bacc.Bacc(None, target_bir_lowering=False, num_devices=NCORES)

    xd = nc.dram_tensor("x", (CIN, 66, 66), F16, kind="ExternalInput")
    hd = nc.dram_tensor("h", (CH, PIX), F16, kind="ExternalInput")
    cd = nc.dram_tensor("c", (CH, PIX), F16, kind="ExternalInput")
    wyd = nc.dram_tensor("wy", (128, 7, 512), F16, kind="ExternalInput")
    wybd = nc.dram_tensor("wyb", (128, 4), F32, kind="ExternalInput")
    wgd = nc.dram_tensor("wg", (128, 16, 512), F16, kind="ExternalInput")
    dwxd = nc.dram_tensor("dwx", (128, 3, 1152), F16, kind="ExternalInput")
    dwid = nc.dram_tensor("dwi", (128, len(DWI_PE), 1152), F16,
                          kind="ExternalInput")
    wivd = nc.dram_tensor("wiv", (128, 9 * len(DWI_DVE)), F32,
                          kind="ExternalInput")
    wxvd = nc.dram_tensor("wxv", (128, 9), F32, kind="ExternalInput")
    xv0d = nc.dram_tensor("xv0", (64, 66, 68), F16, kind="ExternalInput")
    xv1d = nc.dram_tensor("xv1", (64, 66, 68), F16, kind="ExternalInput")
    zd = nc.dram_tensor("zz", (128, 128), F16, kind="ExternalInput")
    ccd = nc.dram_tensor("occ", (CH, PIX), F16, kind="ExternalOutput")
    chd = nc.dram_tensor("och", (CH, PIX), F16, kind="ExternalOutput")

    x_ap, h_ap, c_ap = xd.ap(), hd.ap(), cd.ap()
    cc_ap, ch_ap = ccd.ap(), chd.ap()

    with TileContext(nc) as tc:
        with tc.tile_pool(name="persist", bufs=1) as pp, \
             tc.tile_pool(name="wB", bufs=1) as wB:
            # small zeros tile used to paint halo borders
            zt = pp.tile([128, 68], F16, tag="zt", name="zt")
            nc.sync.dma_start(out=zt[:, 0:68], in_=zd.ap()[:, 0:68])

            # i image, fp16. P0: pixel (r,c) at [r+1, c+2] (so the dx=0 tap
            # slab reads start 4B-aligned). P1 (DVE chunks only): pixel at
            # [r+1, c+1] = P0 shifted left one px, making dx=+-1 taps
            # 4B-aligned too.
            i_p0 = [pp.tile([128, 66, 68], F16, tag=f"ip0{m}", name=f"ip0{m}")
                    for m in range(4)]
            i_p1 = {m: pp.tile([128, 66, 68], F16, tag=f"ip1{m}",
                               name=f"ip1{m}") for m in DWI_DVE}
            for m in range(4):
                nc.scalar.copy(i_p0[m][:, 0, :], zt[:, :])
                nc.scalar.copy(i_p0[m][:, 65, :], zt[:, :])
                nc.scalar.copy(i_p0[m][:, :, 1], zt[:, :66])
                nc.scalar.copy(i_p0[m][:, :, 66], zt[:, :66])
            for m in DWI_DVE:
                nc.scalar.copy(i_p1[m][:, 0, :], zt[:, :])
                nc.scalar.copy(i_p1[m][:, 65, :], zt[:, :])
                nc.scalar.copy(i_p1[m][:, :, 0], zt[:, :66])
                nc.scalar.copy(i_p1[m][:, :, 65], zt[:, :66])

            # ---------------- phase A ----------------
            with (
                tc.tile_pool(name="wA", bufs=1) as wA,
                tc.tile_pool(name="sA", bufs=2) as sA,
                tc.tile_pool(name="psxw", bufs=3, space="PSUM") as psxw,
                tc.tile_pool(name="psi", bufs=4, space="PSUM") as psi,
            ):
                def emit_slab_inputs(n):
                    r0 = 8 * n
                    xv = []
                    if DWX_DVE:
                        for nm, dram in (("xv0", xv0d), ("xv1", xv1d)):
                            t = sA.tile([64, 10, 68], F16, tag=nm, name=nm)
                            nc.sync.dma_start(
                                out=t[:, :, :], in_=dram.ap()[:, r0:r0 + 10, :])
                            xv.append(t)
                    xpads = []
                    npe = 2 if DWX_DVE else 3
                    for ci in range(npe):
                        pc = XCH[ci]
                        xp = sA.tile([128, 10, 66], F16, tag=f"xpad{ci}",
                                     name=f"xpad{ci}")
                        nc.sync.dma_start(
                            out=xp[:pc, :, :],
                            in_=x_ap[128 * ci:128 * ci + pc, r0:r0 + 10, :],
                        )
                        xpads.append(xp)
                    if DWX_DVE:
                        xpads.append(xv)
                    h_sb = []
                    for k in range(4):
                        t = sA.tile([128, 512], F16, tag=f"h{k}", name=f"h{k}")
                        nc.sync.dma_start(
                            out=t[:],
                            in_=h_ap[128 * k:128 * (k + 1), 512 * n:512 * (n + 1)],
                        )
                        h_sb.append(t)
                    return h_sb, xpads

                # startup-critical first: dw-x chunk-0 weights + slab 0
                # inputs so the first matmuls start as early as possible
                dwx_t = wA.tile([128, 3, 1152], F16, tag="dwx", name="dwx")
                wxv_t = wA.tile([128, 9], F32, tag="wxv", name="wxv")
                if DWX_DVE:
                    nc.sync.dma_start(out=wxv_t[:], in_=wxvd.ap())
                nc.sync.dma_start(out=dwx_t[:, 0, :], in_=dwxd.ap()[:, 0, :])
                early = {0: emit_slab_inputs(0)}
                for _ci in (1, 2):
                    nc.sync.dma_start(out=dwx_t[:, _ci, :], in_=dwxd.ap()[:, _ci, :])
                wy_t = wA.tile([128, 7, 512], F16, tag="wy", name="wy")
                for _k in range(7):
                    nc.sync.dma_start(out=wy_t[:, _k, :], in_=wyd.ap()[:, _k, :])
                wyb_t = wA.tile([128, 4], F32, tag="wyb", name="wyb")
                nc.sync.dma_start(out=wyb_t[:], in_=wybd.ap())
                early[1] = emit_slab_inputs(1)
                # prefetch phase-B weights while phase A computes
                wg_t = wB.tile([128, 16, 512], F16, tag="wg", name="wg")
                for _k in range(16):
                    nc.sync.dma_start(out=wg_t[:, _k, :], in_=wgd.ap()[:, _k, :])
                dwi_t = wB.tile([128, len(DWI_PE), 1152], F16, tag="dwi",
                                name="dwi")
                for _ci in range(len(DWI_PE)):
                    nc.sync.dma_start(out=dwi_t[:, _ci, :], in_=dwid.ap()[:, _ci, :])
                wiv_t = wB.tile([128, 9 * len(DWI_DVE)], F32, tag="wiv",
                                name="wiv")
                nc.sync.dma_start(out=wiv_t[:], in_=wivd.ap())

                for n in range(NCHUNK):
                    r0 = 8 * n
                    if n in early:
                        h_sb, xpads = early[n]
                    else:
                        h_sb, xpads = emit_slab_inputs(n)

                    # depthwise 3x3 on x: 9 diag matmuls per chunk -> PSUM
                    # (chunk 2 on the vector engine when DWX_DVE)
                    xw_sb = []
                    npe = 2 if DWX_DVE else 3
                    for ci in range(npe):
                        pc = XCH[ci]
                        ps = psxw.tile([128, 8, 64], F32, tag="psxw", name="psxw")
                        for t, (dy, dx) in enumerate(taps):
                            nc.tensor.matmul(
                                ps[:pc, :, :],
                                dwx_t[:pc, ci, 128 * t:128 * t + pc],
                                xpads[ci][:pc, 1 + dy:9 + dy, 1 + dx:65 + dx],
                                start=(t == 0),
                                stop=(t == 8),
                            )
                        xw = sA.tile([128, 512], F16, tag=f"xw{ci}",
                                     name=f"xw{ci}", bufs=1)
                        nc.scalar.copy(xw[:pc, :], ps[:pc, :, :])
                        xw_sb.append(xw)
                    if DWX_DVE:
                        xv0_t, xv1_t = xpads[2]
                        xaccs = [
                            sA.tile([64, 8, 64], F16, tag=f"xacc{p}",
                                    name=f"xacc{p}")
                            for p in range(2)
                        ]
                        xtmp = sA.tile([64, 8, 64], F16, tag="xtmp",
                                       name="xtmp")
                        for t, (dy, dx) in enumerate(taps):
                            if dx == 0:
                                src = xv0_t[:, 1 + dy:9 + dy, 2:66]
                            elif dx == -1:
                                src = xv1_t[:, 1 + dy:9 + dy, 0:64]
                            else:
                                src = xv1_t[:, 1 + dy:9 + dy, 2:66]
                            w_ap = wxv_t[:64, t:t + 1]
                            if t == 0:
                                nc.vector.tensor_scalar(
                                    out=xaccs[0][:, :, :], in0=src,
                                    scalar1=w_ap, scalar2=None, op0=ALU.mult)
                            else:
                                nc.vector.tensor_scalar(
                                    out=xtmp[:, :, :], in0=src,
                                    scalar1=w_ap, scalar2=None, op0=ALU.mult)
                                nc.vector.tensor_tensor(
                                    out=xaccs[t % 2][:, :, :],
                                    in0=xaccs[(t + 1) % 2][:, :, :],
                                    in1=xtmp[:, :, :], op=ALU.add)
                        xw_sb.append(xaccs[0])  # tap 8 lands in xaccs[0]

                    # i = Wy @ [h; xw] + bias -> i_p0 (and i_p1) interior
                    for m in range(4):
                        ps = psi.tile([128, 512], F32, tag="psi", name="psi")
                        for k in range(4):  # h chunks first (ready earlier)
                            nc.tensor.matmul(
                                ps[:, :],
                                wy_t[:, k, 128 * m:128 * (m + 1)],
                                h_sb[k][:, :],
                                start=(k == 0),
                                stop=False,
                            )
                        for j in range(3):
                            pc = XCH[j]
                            rhs = (xw_sb[j][:, :, :] if (DWX_DVE and j == 2)
                                   else xw_sb[j][:pc, :])
                            nc.tensor.matmul(
                                ps[:, :],
                                wy_t[:pc, 4 + j, 128 * m:128 * (m + 1)],
                                rhs,
                                start=False,
                                stop=(j == 2),
                            )
                        nc.scalar.activation(
                            i_p0[m][:, 1 + r0:9 + r0, 2:66],
                            ps[:, :],
                            AF.Identity,
                            bias=wyb_t[:, m:m + 1],
                            scale=1.0,
                        )
                        if m in DWI_DVE:
                            nc.scalar.activation(
                                i_p1[m][:, 1 + r0:9 + r0, 1:65],
                                ps[:, :],
                                AF.Identity,
                                bias=wyb_t[:, m:m + 1],
                                scale=1.0,
                            )

            # ---------------- phase B ----------------
            with (
                tc.tile_pool(name="sB", bufs=2) as sB,
                tc.tile_pool(name="psb", bufs=2, space="PSUM") as psb,
                tc.tile_pool(name="psg", bufs=3, space="PSUM") as psg,
            ):
                for n in range(NCHUNK):
                    r0 = 8 * n
                    b_sb = [None] * 4
                    # dw-i on tensor engine for chunks in DWI_PE
                    for idx, m in enumerate(DWI_PE):
                        ps = psb.tile([128, 8, 64], F32, tag="psb", name="psb")
                        for t, (dy, dx) in enumerate(taps):
                            nc.tensor.matmul(
                                ps[:, :, :],
                                dwi_t[:, idx, 128 * t:128 * (t + 1)],
                                i_p0[m][:, 1 + r0 + dy:9 + r0 + dy,
                                        2 + dx:66 + dx],
                                start=(t == 0),
                                stop=(t == 8),
                            )
                        bt = sB.tile([128, 8, 64], F16, tag=f"b{m}",
                                     name=f"b{m}")
                        nc.scalar.copy(bt[:, :, :], ps[:, :, :])
                        b_sb[m] = bt
                    # dw-i on vector engine for chunks in DWI_DVE:
                    # per tap: t = in * w[ch]  (tensor_scalar, 4x fp16)
                    #          acc = acc + t   (tensor_tensor, 2x fp16)
                    for di, m in enumerate(DWI_DVE):
                        accs = [
                            sB.tile([128, 8, 64], F16, tag=f"bacc{m}{p}",
                                    name=f"bacc{m}{p}")
                            for p in range(2)
                        ]
                        tmp = sB.tile([128, 8, 64], F16, tag=f"btmp{m}",
                                      name=f"btmp{m}")
                        for t, (dy, dx) in enumerate(taps):
                            if dx == 0:
                                src = i_p0[m][:, 1 + r0 + dy:9 + r0 + dy, 2:66]
                            elif dx == -1:
                                src = i_p1[m][:, 1 + r0 + dy:9 + r0 + dy, 0:64]
                            else:
                                src = i_p1[m][:, 1 + r0 + dy:9 + r0 + dy, 2:66]
                            w_ap = wiv_t[:, 9 * di + t:9 * di + t + 1]
                            if t == 0:
                                nc.vector.tensor_scalar(
                                    out=accs[0][:, :, :], in0=src,
                                    scalar1=w_ap, scalar2=None, op0=ALU.mult)
                            else:
                                nc.vector.tensor_scalar(
                                    out=tmp[:, :, :], in0=src,
                                    scalar1=w_ap, scalar2=None, op0=ALU.mult)
                                nc.vector.tensor_tensor(
                                    out=accs[t % 2][:, :, :],
                                    in0=accs[(t + 1) % 2][:, :, :],
                                    in1=tmp[:, :, :], op=ALU.add)
                        b_sb[m] = accs[0]  # tap 8 lands in accs[0]

                    for m in range(4):
                        c_t = sB.tile([128, 512], F16, tag="c", name="c")
                        nc.sync.dma_start(
                            out=c_t[:],
                            in_=c_ap[128 * m:128 * (m + 1), 512 * n:512 * (n + 1)],
                        )
                        # gate order in wg packing: pair0 = (i, f) both
                        # sigmoid; pair1 = (o, c) sigmoid + relu
                        pstiles = []
                        for gp in range(2):
                            ps = psg.tile([128, 2, 512], F32, tag="psg",
                                          name="psg")
                            for gg in range(2):
                                g = 2 * gp + gg
                                for k in range(4):
                                    nc.tensor.matmul(
                                        ps[:, gg, :],
                                        wg_t[:, 4 * g + k, 128 * m:128 * (m + 1)],
                                        b_sb[k][:, :, :],
                                        start=(k == 0),
                                        stop=(k == 3),
                                    )
                            pstiles.append(ps)
                        sif = sB.tile([128, 2, 512], F16, tag="sif", name="sif")
                        nc.scalar.activation(sif[:, :, :], pstiles[0][:, :, :],
                                             AF.Sigmoid)
                        so = sB.tile([128, 512], F16, tag="so", name="so")
                        nc.scalar.activation(so[:, :], pstiles[1][:, 0, :],
                                             AF.Sigmoid)
                        gcr = sB.tile([128, 512], F16, tag="gcr", name="gcr")
                        nc.scalar.activation(gcr[:, :], pstiles[1][:, 1, :],
                                             AF.Relu)

                        # pointwise: cc = f*c + i*min(gc,6); ch = o*clip(cc,0,6)
                        # 4 DVE ops (two STT fusions); the relu(cc) runs on
                        # the scalar engine, which has slack in phase B
                        u = sB.tile([128, 512], F16, tag="u", name="u")
                        nc.vector.scalar_tensor_tensor(
                            out=u[:, :], in0=gcr[:, :], scalar=6.0,
                            in1=sif[:, 0, :], op0=ALU.min, op1=ALU.mult)
                        tv = sB.tile([128, 512], F16, tag="tv", name="tv")
                        if TV_ON_GP:
                            nc.gpsimd.tensor_tensor(
                                out=tv[:, :], in0=sif[:, 1, :],
                                in1=c_t[:, :], op=ALU.mult)
                        else:
                            nc.vector.tensor_mul(tv[:, :], sif[:, 1, :],
                                                 c_t[:, :])
                        cc_t = sB.tile([128, 512], F16, tag="cc", name="cc")
                        nc.vector.tensor_add(cc_t[:, :], u[:, :], tv[:, :])
                        nc.sync.dma_start(
                            out=cc_ap[128 * m:128 * (m + 1), 512 * n:512 * (n + 1)],
                            in_=cc_t[:],
                        )
                        rcc = sB.tile([128, 512], F16, tag="rcc", name="rcc")
                        nc.scalar.activation(rcc[:, :], cc_t[:, :], AF.Relu)
                        ch_t = sB.tile([128, 512], F16, tag="ch", name="ch")
                        nc.vector.scalar_tensor_tensor(
                            out=ch_t[:, :], in0=rcc[:, :], scalar=6.0,
                            in1=so[:, :], op0=ALU.min, op1=ALU.mult)
                        nc.sync.dma_start(
                            out=ch_ap[128 * m:128 * (m + 1), 512 * n:512 * (n + 1)],
                            in_=ch_t[:],
                        )

    nc.compile()
    return nc


def pack_weights(W_dw, W_dwb, Wy, Wy_b, Wi, Wbi, Wbf, Wbc, Wbo):
    WyT = Wy[:, :, 0, 0].T.astype(np.float32)  # (832, 512) lhsT
    wy = np.zeros((128, 7, 512), np.float32)
    for k in range(4):  # h chunks first
        wy[:, k, :] = WyT[320 + 128 * k:320 + 128 * (k + 1), :]
    for k in range(2):
        wy[:, 4 + k, :] = WyT[128 * k:128 * (k + 1), :]
    wy[:64, 6, :] = WyT[256:320, :]

    wyb = (Wy_b + Wy[:, :320, 0, 0] @ W_dwb).astype(np.float32)
    wyb = wyb.reshape(4, 128).T.copy()

    def diag_pack(Wtaps, nch, chunks):
        out = np.zeros((128, len(chunks), 1152), np.float32)
        w = Wtaps[:, 0].reshape(nch, 9)  # (nch, 9) tap-major (dy,dx)
        for oi, ci in enumerate(chunks):
            pc = min(128, nch - 128 * ci)
            for t in range(9):
                idx = np.arange(pc)
                out[idx, oi, 128 * t + idx] = w[128 * ci + idx, t]
        return out

    dwx = diag_pack(W_dw, CIN, (0, 1, 2))
    dwi = diag_pack(Wi, CH, DWI_PE)

    wiv = np.zeros((128, 9 * len(DWI_DVE)), np.float32)
    wi9 = Wi[:, 0].reshape(CH, 9)
    for di, m in enumerate(DWI_DVE):
        wiv[:, 9 * di:9 * (di + 1)] = wi9[128 * m:128 * (m + 1), :]

    wxv = np.zeros((128, 9), np.float32)
    wxv[:64, :] = W_dw[256:320, 0].reshape(64, 9)

    # gate order (i, f, o, c): pair0 sigmoid-sigmoid, pair1 sigmoid-relu
    wg = np.zeros((128, 16, 512), np.float32)
    for g, W in enumerate([Wbi, Wbf, Wbo, Wbc]):
        lhsT = W[:, :, 0, 0].T.astype(np.float32)  # (512 in, 512 out)
        for k in range(4):
            wg[:, 4 * g + k, :] = lhsT[128 * k:128 * (k + 1), :]

    f16 = lambda a: np.ascontiguousarray(a, dtype=np.float16)
    return {
        "wy": f16(wy), "wyb": np.ascontiguousarray(wyb), "wg": f16(wg),
        "dwx": f16(dwx), "dwi": f16(dwi), "wiv": np.ascontiguousarray(wiv),
        "wxv": np.ascontiguousarray(wxv),
    }


_CACHE = {}


def _get_nc():
    if "nc" not in _CACHE:
        _CACHE["nc"] = build_nc()
    return _CACHE["nc"]


def run(inputs, trace=False, tmpdir=None):
    """inputs: dict as from setup_inputs(). Returns ((ch, cc), results_obj)."""
    inp = {k: np.asarray(v, np.float32) for k, v in inputs.items()}
    packed = pack_weights(
        inp["W_dw"], inp["W_dwb"], inp["Wy"], inp["Wy_b"], inp["Wi"],
        inp["Wbi"], inp["Wbf"], inp["Wbc"], inp["Wbo"],
    )
    xpad_host = np.zeros((B, CIN, 66, 66), np.float16)
    xpad_host[:, :, 1:65, 1:65] = inp["x"]
    # dw-x DVE chunk: x channels 256:320 in the aligned P0/P1 layouts
    xv0_host = np.zeros((B, 64, 66, 68), np.float16)
    xv0_host[:, :, 1:65, 2:66] = inp["x"][:, 256:320]
    xv1_host = np.zeros((B, 64, 66, 68), np.float16)
    xv1_host[:, :, 1:65, 1:65] = inp["x"][:, 256:320]
    h_host = inp["h"].reshape(B, CH, PIX).astype(np.float16)
    c_host = inp["c"].reshape(B, CH, PIX).astype(np.float16)
    in_maps = []
    for b in range(B):
        in_maps.append({
            "x": xpad_host[b],
            "xv0": xv0_host[b],
            "xv1": xv1_host[b],
            "h": np.ascontiguousarray(h_host[b]),
            "c": np.ascontiguousarray(c_host[b]),
            "zz": np.zeros((128, 128), np.float16),
            **packed,
        })
    nc = _get_nc()
    kwargs = {}
    if trace:
        _enable_trace_hooks()
        kwargs = dict(trace=True, trace_cores=[0])
        if tmpdir:
            kwargs["tmpdir"] = tmpdir
    res = run_bass_kernel_spmd(nc, in_maps, core_ids=list(range(NCORES)), **kwargs)
    ch = np.stack([res.results[b]["och"].reshape(CH, HW, HW) for b in range(B)])
    cc = np.stack([res.results[b]["occ"].reshape(CH, HW, HW) for b in range(B)])
    return (ch.astype(np.float32), cc.astype(np.float32)), res


def kernel(**inputs):
    (ch, cc), _ = run(inputs, trace=False)
    return ch, cc


# ---------- optional NTFF tracing support (test harness only) ----------

def _enable_trace_hooks():
    import types, ctypes, contextlib
    if "antenv.axon_hooks" in sys.modules:
        return
    import concourse.bass_utils as bass_utils

    def _ntff_profile_via_ctypes(so_path):
        lib = ctypes.CDLL(so_path)
        if not hasattr(lib, "axon_start_nrt_profile"):
            return None
        lib.axon_start_nrt_profile.argtypes = [
            ctypes.POINTER(ctypes.c_int64), ctypes.c_size_t]
        lib.axon_start_nrt_profile.restype = ctypes.c_int64
        lib.axon_stop_nrt_profile.argtypes = [ctypes.c_char_p]
        lib.axon_stop_nrt_profile.restype = ctypes.c_int64

        @contextlib.contextmanager
        def _hook(output_dir, device_ids):
            import jax
            jax.devices()
            if device_ids:
                ids = (ctypes.c_int64 * len(device_ids))(*device_ids)
                rc = lib.axon_start_nrt_profile(ids, len(device_ids))
            else:
                rc = lib.axon_start_nrt_profile(None, 0)
            if rc != 0:
                raise RuntimeError(f"axon_start_nrt_profile rc={rc}")
            try:
                yield
            finally:
                lib.axon_stop_nrt_profile(str(output_dir).encode())
        return _hook

    hook = _ntff_profile_via_ctypes("/opt/axon/libaxon_pjrt.so")
    mod = types.ModuleType("antenv.axon_hooks")
    mod.get_axon_ntff_profile_hook = lambda: hook
    mod.set_axon_ntff_profile_hook = lambda h: None
    sys.modules["antenv.axon_hooks"] = mod
    bass_utils.upload_artifacts = lambda tmpdir: "local://" + str(tmpdir)
